# revision 14
# baseline (speedup 1.0000x reference)
"""MeshGraphNet on 8 Trainium2 NeuronCores (Bass/Tile SPMD kernel).

Sharding: nodes in contiguous blocks of 12500/core (padded to 12544); edges
sharded by dst core and bucketed into 128-node blocks so the segment-sum is
core-local. Per message-passing block each core computes Za = nf @ W1_src for
its node slice; an AllGather replicates the table, and edge tiles gather rows
of it by src (indirect DMA). The dst contribution gathers from the local Zb
table. The first edge-MLP layer is assembled in PSUM via transpose-as-matmul
accumulation; the segment-sum uses a one-hot (is_equal vs iota) matmul per
128-edge tile. LayerNorm means are folded into last-layer weights
(W3' = W3 @ (I - J/H)); LN gains/biases must be trivial (they are in this
model family).
"""
import numpy as np
import ml_dtypes

import concourse.bass as bass
import concourse.bacc as bacc
import concourse.mybir as mybir
import concourse.tile as tile

BF16 = ml_dtypes.bfloat16
NCORES = 8
N, E, H = 100000, 800000, 128
IN_N, IN_E, OUT, P_BLOCKS = 4, 3, 2, 15
EPS = 1e-5
NPC_RAW = N // NCORES            # 12500
NB = 98                          # 128-node blocks per core
NPC = NB * 128                   # 12544
UNROLL = 7                       # node-blocks per For_i iteration (98 = 14*7)

F32 = mybir.dt.float32
BF = mybir.dt.bfloat16
I32 = mybir.dt.int32

_CACHE = {}


# --------------------------------------------------------------------------
# device program
# --------------------------------------------------------------------------

def build_nc(BT, nblocks=P_BLOCKS, dbg=False):
    """BT: edge tiles (of 128) per 128-node block; divisible by 3."""
    ECP = NB * BT * 128
    NSLOT = P_BLOCKS * 9 + 4 + 3
    nc = bacc.Bacc(num_devices=NCORES)

    nfeat = nc.declare_dram_parameter("nfeat", [NPC, IN_N], F32, isOutput=False)
    efeat = nc.declare_dram_parameter("efeat", [ECP, IN_E], F32, isOutput=False)
    srcmap = nc.declare_dram_parameter("srcmap", [ECP], I32, isOutput=False)
    dstloc = nc.declare_dram_parameter("dstloc", [ECP], I32, isOutput=False)
    dstrel = nc.declare_dram_parameter("dstrel", [ECP], F32, isOutput=False)
    Wmm = nc.declare_dram_parameter("Wmm", [128, NSLOT * 128], BF, isOutput=False)
    We1 = nc.declare_dram_parameter("We1", [IN_E, 128], BF, isOutput=False)
    Wn1 = nc.declare_dram_parameter("Wn1", [IN_N, 128], BF, isOutput=False)
    Bias = nc.declare_dram_parameter("Bias", [128, 100], F32, isOutput=False)
    identp = nc.declare_dram_parameter("ident", [128, 128], BF, isOutput=False)
    iotap = nc.declare_dram_parameter("iota", [128, 128], F32, isOutput=False)
    out_col = nc.declare_dram_parameter("out_col", [OUT, NPC], F32, isOutput=True)
    if dbg:
        dbg_nf = nc.declare_dram_parameter("dbg_nf", [128, NPC], F32, isOutput=True)
        dbg_za = nc.declare_dram_parameter("dbg_za", [NPC, H], F32, isOutput=True)
        dbg_zb = nc.declare_dram_parameter("dbg_zb", [NPC, H], F32, isOutput=True)
        dbg_ef = nc.declare_dram_parameter("dbg_ef", [128, 9 * 128], F32, isOutput=True)
        dbg_agg = nc.declare_dram_parameter("dbg_agg", [128, 128], F32, isOutput=True)
        dbg_nf1 = nc.declare_dram_parameter("dbg_nf1", [128, NPC], F32, isOutput=True)
        dbg_ef1 = nc.declare_dram_parameter("dbg_ef1", [128, 9 * 128], F32, isOutput=True)
        dbg_g = nc.declare_dram_parameter("dbg_g", [128, 384], F32, isOutput=True)
        dbg_x = nc.declare_dram_parameter("dbg_x", [128, 384], F32, isOutput=True)
        dbg_h1 = nc.declare_dram_parameter("dbg_h1", [128, 384], F32, isOutput=True)
        dbg_gt = nc.declare_dram_parameter("dbg_gt", [128, 128], F32, isOutput=True)
        dbg_wc = nc.declare_dram_parameter("dbg_wc", [128, 128], F32, isOutput=True)

    ef_dram = nc.dram_tensor("ef_state", [ECP, H], F32)
    za_own = nc.dram_tensor("za_own", [NPC, H], BF)
    zb_loc = nc.dram_tensor("zb_loc", [NPC, H], BF)
    za_full = nc.dram_tensor("za_full", [NCORES * NPC, H], BF, addr_space="Shared")

    def wslot(i):
        return slice(i * 128, (i + 1) * 128)

    W1a = lambda k: wslot(k * 9 + 0)
    W1b = lambda k: wslot(k * 9 + 1)
    W1c = lambda k: wslot(k * 9 + 2)
    W2s = lambda k: wslot(k * 9 + 3)
    W3p = lambda k: wslot(k * 9 + 4)
    Wn1a = lambda k: wslot(k * 9 + 5)
    Wn1b = lambda k: wslot(k * 9 + 6)
    Wn2 = lambda k: wslot(k * 9 + 7)
    Wn3p = lambda k: wslot(k * 9 + 8)
    base = P_BLOCKS * 9
    We2, We3p, Wne2, Wne3p = (wslot(base + i) for i in range(4))
    Wd1, Wd2, Wd3 = (wslot(base + 4 + i) for i in range(3))

    def bcol(i):
        return slice(i, i + 1)
    EB1 = lambda k: bcol(k * 6 + 0)
    EB2 = lambda k: bcol(k * 6 + 1)
    EC3 = lambda k: bcol(k * 6 + 2)
    NB1 = lambda k: bcol(k * 6 + 3)
    NB2 = lambda k: bcol(k * 6 + 4)
    NC3 = lambda k: bcol(k * 6 + 5)
    bb = P_BLOCKS * 6
    EEB1, EEB2, EEC3, NEB1, NEB2, NEC3, DB1, DB2, DB3 = (bcol(bb + i) for i in range(9))

    from contextlib import ExitStack
    with tile.TileContext(nc) as tc, ExitStack() as es:
        cp = es.enter_context(tc.tile_pool(name="consts", bufs=1))
        W_sb = cp.tile([128, NSLOT * 128], BF)
        We1_sb = cp.tile([IN_E, 128], BF)
        Wn1_sb = cp.tile([IN_N, 128], BF)
        B_sb = cp.tile([128, 100], F32)
        id_sb = cp.tile([128, 128], BF)
        io_sb = cp.tile([128, 128], F32)
        nf_sb = cp.tile([128, NPC], F32)
        oc_sb = cp.tile([OUT, NPC], F32)
        nc.sync.dma_start(out=W_sb[:], in_=Wmm[:, :])
        nc.sync.dma_start(out=We1_sb[:], in_=We1[:, :])
        nc.sync.dma_start(out=Wn1_sb[:], in_=Wn1[:, :])
        nc.sync.dma_start(out=B_sb[:], in_=Bias[:, :])
        nc.sync.dma_start(out=id_sb[:], in_=identp[:, :])
        nc.sync.dma_start(out=io_sb[:], in_=iotap[:, :])
        nc.vector.memset(oc_sb[:], 0.0)

        sp = es.enter_context(tc.tile_pool(name="work", bufs=2))
        sp3 = es.enter_context(tc.tile_pool(name="work3", bufs=3))
        pp = es.enter_context(tc.tile_pool(name="psum", bufs=1, space="PSUM"))
        pp2 = es.enter_context(tc.tile_pool(name="psum2", bufs=2, space="PSUM"))

        ident = id_sb[:]
        ALL_ENG = tuple(nc.engines)

        def ln_tail(CC, c3col, scaled_out, w):
            """psum CC f32 [128, w] col-major pre-LN -> scaled_out f32 row-major
            (LN applied; trivial gain/bias)."""
            nt = w // 128
            h3T = sp.tile([128, w], BF, tag="h3T")
            nc.scalar.activation(h3T[:, :w], CC[:, :w],
                                 mybir.ActivationFunctionType.Identity,
                                 bias=B_sb[:, c3col])
            YF = pp.tile([128, w], BF, tag="pF", space="PSUM")
            for t in range(nt):
                ts = slice(t * 128, (t + 1) * 128)
                nc.tensor.transpose(YF[:, ts], h3T[:, ts], ident)
            scr = sp.tile([128, w], BF, tag="scr")
            stat = sp.tile([128, 4], F32, tag="stat")
            for t in range(nt):
                ts = slice(t * 128, (t + 1) * 128)
                nc.scalar.activation(scr[:, ts], YF[:, ts],
                                     mybir.ActivationFunctionType.Square,
                                     accum_out=stat[:, t:t + 1])
            sd = sp.tile([128, 4], F32, tag="sd")
            nc.scalar.activation(sd[:, :nt], stat[:, :nt],
                                 mybir.ActivationFunctionType.Sqrt,
                                 scale=1.0 / 128.0, bias=B_sb[:, 99:100])
            rstd = sp.tile([128, 4], F32, tag="rstd")
            nc.vector.reciprocal(rstd[:, :nt], sd[:, :nt])
            for t in range(nt):
                ts = slice(t * 128, (t + 1) * 128)
                nc.vector.tensor_scalar(scaled_out[:, ts], YF[:, ts],
                                        rstd[:, t:t + 1], None,
                                        mybir.AluOpType.mult)

        def node_mlp(b_nodecol, blk, aggE):
            aggT = sp.tile([128, 128], BF, tag="aggT")
            nc.scalar.copy(out=aggT[:], in_=aggE[:])
            nfbf = sp.tile([128, 128], BF, tag="nfbf")
            nc.vector.tensor_copy(out=nfbf[:], in_=nf_sb[:, b_nodecol])
            TP = pp.tile([128, 128], BF, tag="pD", space="PSUM")
            nc.tensor.transpose(TP[:], nfbf[:], ident)
            nfT = sp.tile([128, 128], BF, tag="nfT")
            nc.scalar.copy(out=nfT[:], in_=TP[:])
            FF = pp.tile([128, 128], F32, tag="pA", space="PSUM")
            nc.tensor.matmul(FF[:], lhsT=W_sb[:, Wn1a(blk)], rhs=aggT[:],
                             start=True, stop=False)
            nc.tensor.matmul(FF[:], lhsT=W_sb[:, Wn1b(blk)], rhs=nfT[:],
                             start=False, stop=True)
            n1 = sp.tile([128, 128], BF, tag="n1")
            nc.scalar.activation(n1[:], FF[:], mybir.ActivationFunctionType.Relu,
                                 bias=B_sb[:, NB1(blk)])
            F2 = pp.tile([128, 128], F32, tag="pB", space="PSUM")
            nc.tensor.matmul(F2[:], lhsT=W_sb[:, Wn2(blk)], rhs=n1[:],
                             start=True, stop=True)
            n2 = sp.tile([128, 128], BF, tag="n2")
            nc.scalar.activation(n2[:], F2[:], mybir.ActivationFunctionType.Relu,
                                 bias=B_sb[:, NB2(blk)])
            F3 = pp.tile([128, 128], F32, tag="pC", space="PSUM")
            nc.tensor.matmul(F3[:], lhsT=W_sb[:, Wn3p(blk)], rhs=n2[:],
                             start=True, stop=True)
            scaled = sp.tile([128, 128], F32, tag="nscaled")
            ln_tail(F3, NC3(blk), scaled, 128)
            newnf = sp.tile([128, 128], F32, tag="newnf")
            nc.vector.tensor_tensor(out=newnf[:], in0=scaled[:],
                                    in1=nf_sb[:, b_nodecol], op=mybir.AluOpType.add)
            nc.vector.tensor_copy(out=nf_sb[:, b_nodecol], in_=newnf[:])
            return newnf

        def enc_node_mlp(b_nodecol, b):
            nfe = sp.tile([128, IN_N], F32, tag="nfe")
            nc.sync.dma_start(
                out=nfe[:],
                in_=nfeat.ap().rearrange("(b p) f -> p b f", p=128)[
                    :, bass.ds(b, 1), :])
            nfeb = sp.tile([128, IN_N], BF, tag="nfeb")
            nc.vector.tensor_copy(out=nfeb[:], in_=nfe[:])
            TN = pp.tile([IN_N, 128], BF, tag="pD", space="PSUM")
            nc.tensor.transpose(TN[:], nfeb[:], ident)
            fnT = sp.tile([IN_N, 128], BF, tag="fnT")
            nc.scalar.copy(out=fnT[:], in_=TN[:])
            A = pp.tile([128, 128], F32, tag="pA", space="PSUM")
            nc.tensor.matmul(A[:], lhsT=Wn1_sb[:], rhs=fnT[:],
                             start=True, stop=True)
            h1 = sp.tile([128, 128], BF, tag="n1")
            nc.scalar.activation(h1[:], A[:], mybir.ActivationFunctionType.Relu,
                                 bias=B_sb[:, NEB1])
            B2 = pp.tile([128, 128], F32, tag="pB", space="PSUM")
            nc.tensor.matmul(B2[:], lhsT=W_sb[:, Wne2], rhs=h1[:],
                             start=True, stop=True)
            h2 = sp.tile([128, 128], BF, tag="n2")
            nc.scalar.activation(h2[:], B2[:], mybir.ActivationFunctionType.Relu,
                                 bias=B_sb[:, NEB2])
            C3 = pp.tile([128, 128], F32, tag="pC", space="PSUM")
            nc.tensor.matmul(C3[:], lhsT=W_sb[:, Wne3p], rhs=h2[:],
                             start=True, stop=True)
            nnf = sp.tile([128, 128], F32, tag="newnf")
            ln_tail(C3, NEC3, nnf, 128)
            nc.vector.tensor_copy(out=nf_sb[:, b_nodecol], in_=nnf[:])
            return nnf

        def make_tables(newnf, b_rowbase, blk_next):
            nbf = sp.tile([128, 128], BF, tag="nbf")
            nc.vector.tensor_copy(out=nbf[:], in_=newnf[:])
            TP2 = pp.tile([128, 128], BF, tag="pD", space="PSUM")
            nc.tensor.transpose(TP2[:], nbf[:], ident)
            nfT2 = sp.tile([128, 128], BF, tag="nfT2")
            nc.scalar.copy(out=nfT2[:], in_=TP2[:])
            ZP = pp.tile([128, 128], F32, tag="pA", space="PSUM")
            nc.tensor.matmul(ZP[:], lhsT=nfT2[:], rhs=W_sb[:, W1a(blk_next)],
                             start=True, stop=True)
            zab = sp.tile([128, 128], BF, tag="zab")
            nc.scalar.copy(out=zab[:], in_=ZP[:])
            nc.sync.dma_start(out=za_own[bass.ds(b_rowbase, 128), :], in_=zab[:])
            ZP2 = pp.tile([128, 128], F32, tag="pB", space="PSUM")
            nc.tensor.matmul(ZP2[:], lhsT=nfT2[:], rhs=W_sb[:, W1b(blk_next)],
                             start=True, stop=True)
            zbb = sp.tile([128, 128], BF, tag="zbb")
            nc.scalar.copy(out=zbb[:], in_=ZP2[:])
            nc.sync.dma_start(out=zb_loc[bass.ds(b_rowbase, 128), :], in_=zbb[:])

        def decoder(newnf, b_nodecol):
            nbf = sp.tile([128, 128], BF, tag="nbf")
            nc.vector.tensor_copy(out=nbf[:], in_=newnf[:])
            TP2 = pp.tile([128, 128], BF, tag="pD", space="PSUM")
            nc.tensor.transpose(TP2[:], nbf[:], ident)
            nfT2 = sp.tile([128, 128], BF, tag="nfT2")
            nc.scalar.copy(out=nfT2[:], in_=TP2[:])
            D1 = pp.tile([128, 128], F32, tag="pA", space="PSUM")
            nc.tensor.matmul(D1[:], lhsT=W_sb[:, Wd1], rhs=nfT2[:],
                             start=True, stop=True)
            d1 = sp.tile([128, 128], BF, tag="n1")
            nc.scalar.activation(d1[:], D1[:], mybir.ActivationFunctionType.Relu,
                                 bias=B_sb[:, DB1])
            D2 = pp.tile([128, 128], F32, tag="pB", space="PSUM")
            nc.tensor.matmul(D2[:], lhsT=W_sb[:, Wd2], rhs=d1[:],
                             start=True, stop=True)
            d2 = sp.tile([128, 128], BF, tag="n2")
            nc.scalar.activation(d2[:], D2[:], mybir.ActivationFunctionType.Relu,
                                 bias=B_sb[:, DB2])
            D3 = pp.tile([OUT, 128], F32, tag="pC", space="PSUM")
            nc.tensor.matmul(D3[:], lhsT=W_sb[:, Wd3][:, :OUT], rhs=d2[:],
                             start=True, stop=True)
            nc.scalar.activation(oc_sb[:, b_nodecol], D3[:],
                                 mybir.ActivationFunctionType.Identity,
                                 bias=B_sb[:OUT, DB3])

        # ---------------- encoder: edges ----------------
        with tc.For_i(0, NB // UNROLL, 1, hint_engines=ALL_ENG, name="ee") as ig:
            for ib in range(UNROLL):
                b = ig * UNROLL + ib
                feat = sp.tile([128, BT * IN_E], F32, tag="efeat")
                nc.sync.dma_start(
                    out=feat[:],
                    in_=efeat.ap().rearrange("(e p) f -> p e f", p=128)[
                        :, bass.ds(b * BT, BT), :])
                fbf = sp.tile([128, BT * IN_E], BF, tag="efeatb")
                nc.vector.tensor_copy(out=fbf[:], in_=feat[:])
                for g in range(BT // 3):
                    TE = pp.tile([IN_E, 384], BF, tag="pD", space="PSUM")
                    for t in range(3):
                        tt = g * 3 + t
                        nc.tensor.transpose(TE[:, t * 128:(t + 1) * 128],
                                            fbf[:, tt * IN_E:(tt + 1) * IN_E],
                                            ident)
                    feT = sp.tile([IN_E, 384], BF, tag="feT")
                    nc.scalar.copy(out=feT[:], in_=TE[:])
                    A = pp.tile([128, 384], F32, tag="pA", space="PSUM")
                    nc.tensor.matmul(A[:], lhsT=We1_sb[:], rhs=feT[:],
                                     start=True, stop=True)
                    h1 = sp.tile([128, 384], BF, tag="H1")
                    nc.scalar.activation(h1[:], A[:],
                                         mybir.ActivationFunctionType.Relu,
                                         bias=B_sb[:, EEB1])
                    B2 = pp.tile([128, 384], F32, tag="pB", space="PSUM")
                    nc.tensor.matmul(B2[:], lhsT=W_sb[:, We2], rhs=h1[:],
                                     start=True, stop=True)
                    h2 = sp.tile([128, 384], BF, tag="H2")
                    nc.scalar.activation(h2[:], B2[:],
                                         mybir.ActivationFunctionType.Relu,
                                         bias=B_sb[:, EEB2])
                    C3 = pp.tile([128, 384], F32, tag="pC", space="PSUM")
                    nc.tensor.matmul(C3[:], lhsT=W_sb[:, We3p], rhs=h2[:],
                                     start=True, stop=True)
                    newef = sp.tile([128, 384], F32, tag="newef")
                    ln_tail(C3, EEC3, newef, 384)
                    nc.sync.dma_start(
                        out=ef_dram.ap().rearrange("(e p) f -> p e f", p=128)[
                            :, bass.ds(b * BT + g * 3, 3), :],
                        in_=newef[:].rearrange("p (e f) -> p e f", f=128))

        # ---------------- encoder: nodes + Za/Zb(0) ----------------
        with tc.For_i(0, NB // UNROLL, 1, hint_engines=ALL_ENG, name="en") as ig:
            for ib in range(UNROLL):
                b = ig * UNROLL + ib
                nnf = enc_node_mlp(bass.ds(b * 128, 128), b)
                make_tables(nnf, b * 128, 0)

        if dbg:
            nc.sync.dma_start(out=dbg_nf[:, :], in_=nf_sb[:])
            for rr in range(0, NPC, 1792):
                zz = sp.tile([128, 1792 // 128 * H], F32, tag="zdump")
                nc.gpsimd.dma_start(out=zz[:], in_=za_own.ap().rearrange(
                    "(e p) f -> p e f", p=128)[:, rr // 128:(rr + 1792) // 128, :])
                nc.sync.dma_start(out=dbg_za.ap().rearrange(
                    "(e p) f -> p e f", p=128)[:, rr // 128:(rr + 1792) // 128, :], in_=zz[:])
                zz2 = sp.tile([128, 1792 // 128 * H], F32, tag="zdump")
                nc.gpsimd.dma_start(out=zz2[:], in_=zb_loc.ap().rearrange(
                    "(e p) f -> p e f", p=128)[:, rr // 128:(rr + 1792) // 128, :])
                nc.sync.dma_start(out=dbg_zb.ap().rearrange(
                    "(e p) f -> p e f", p=128)[:, rr // 128:(rr + 1792) // 128, :], in_=zz2[:])
            ee_ = sp.tile([128, 9 * 128], F32, tag="edump")
            nc.sync.dma_start(out=ee_[:], in_=ef_dram.ap().rearrange(
                "(e p) f -> p e f", p=128)[:, 0:9, :])
            nc.sync.dma_start(out=dbg_ef[:, :], in_=ee_[:])

        # ---------------- message-passing blocks ----------------
        for blk in range(nblocks):
            nc.gpsimd.collective_compute(
                "AllGather", mybir.AluOpType.bypass,
                replica_groups=[list(range(NCORES))],
                ins=[za_own[:, :]], outs=[za_full[:, :]])

            with tc.For_i(0, NB // UNROLL, 1, hint_engines=ALL_ENG,
                          name=f"blk{blk}") as ig:
                for ib in range(UNROLL):
                    b = ig * UNROLL + ib
                    isrc = sp.tile([128, BT], I32, tag="isrc")
                    nc.sync.dma_start(out=isrc[:], in_=srcmap.ap().rearrange(
                        "(t p) -> p t", p=128)[:, bass.ds(b * BT, BT)])
                    idst = sp.tile([128, BT], I32, tag="idst")
                    nc.sync.dma_start(out=idst[:], in_=dstloc.ap().rearrange(
                        "(t p) -> p t", p=128)[:, bass.ds(b * BT, BT)])
                    drel = sp.tile([128, BT], F32, tag="drel")
                    nc.sync.dma_start(out=drel[:], in_=dstrel.ap().rearrange(
                        "(t p) -> p t", p=128)[:, bass.ds(b * BT, BT)])

                    aggE = pp2.tile([128, 128], F32, tag="agg", space="PSUM")

                    for g in range(BT // 3):
                        G3 = sp3.tile([128, 384], BF, tag="G3")
                        X3 = sp3.tile([128, 384], BF, tag="X3")
                        for t in range(3):
                            tt = g * 3 + t
                            ts = slice(t * 128, (t + 1) * 128)
                            nc.gpsimd.indirect_dma_start(
                                out=G3[:, ts], out_offset=None, in_=za_full[:, :],
                                in_offset=bass.IndirectOffsetOnAxis(
                                    ap=isrc[:, tt:tt + 1], axis=0))
                            nc.gpsimd.indirect_dma_start(
                                out=X3[:, ts], out_offset=None, in_=zb_loc[:, :],
                                in_offset=bass.IndirectOffsetOnAxis(
                                    ap=idst[:, tt:tt + 1], axis=0))
                        ef3 = sp3.tile([128, 384], F32, tag="ef3")
                        nc.sync.dma_start(
                            out=ef3[:],
                            in_=ef_dram.ap().rearrange("(e p) f -> p e f", p=128)[
                                :, bass.ds(b * BT + g * 3, 3), :])
                        efb = sp.tile([128, 384], BF, tag="efb")
                        nc.vector.tensor_copy(out=efb[:], in_=ef3[:])
                        TD = pp.tile([128, 384], BF, tag="pD", space="PSUM")
                        for t in range(3):
                            ts = slice(t * 128, (t + 1) * 128)
                            nc.tensor.transpose(TD[:, ts], efb[:, ts], ident)
                        efT = sp.tile([128, 384], BF, tag="efT")
                        nc.scalar.copy(out=efT[:], in_=TD[:])
                        A = pp.tile([128, 384], F32, tag="pA", space="PSUM")
                        for t in range(3):
                            ts = slice(t * 128, (t + 1) * 128)
                            nc.tensor.matmul(A[:, ts], lhsT=G3[:, ts], rhs=ident,
                                             start=True, stop=False)
                            nc.tensor.matmul(A[:, ts], lhsT=X3[:, ts], rhs=ident,
                                             start=False, stop=False)
                            nc.tensor.matmul(A[:, ts], lhsT=W_sb[:, W1c(blk)],
                                             rhs=efT[:, ts],
                                             start=False, stop=True)
                        h1 = sp.tile([128, 384], BF, tag="H1")
                        nc.scalar.activation(h1[:], A[:],
                                             mybir.ActivationFunctionType.Relu,
                                             bias=B_sb[:, EB1(blk)])
                        if dbg and blk == 0 and ib == 0 and g == 0:
                            nc.gpsimd.dma_start(out=dbg_g[:, :], in_=G3[:])
                            nc.gpsimd.dma_start(out=dbg_x[:, :], in_=X3[:])
                            nc.gpsimd.dma_start(out=dbg_h1[:, :], in_=h1[:])
                            AD = pp.tile([128, 128], F32, tag="pdbg", space="PSUM")
                            nc.tensor.matmul(AD[:], lhsT=G3[:, 0:128], rhs=ident,
                                             start=True, stop=True)
                            gt_sb = sp.tile([128, 128], F32, tag="gtdump")
                            nc.vector.tensor_copy(out=gt_sb[:], in_=AD[:])
                            nc.gpsimd.dma_start(out=dbg_gt[:, :], in_=gt_sb[:])
                            AD2 = pp.tile([128, 128], F32, tag="pdbg", space="PSUM")
                            nc.tensor.matmul(AD2[:], lhsT=W_sb[:, W1c(blk)],
                                             rhs=efT[:, 0:128], start=True, stop=True)
                            wc_sb = sp.tile([128, 128], F32, tag="gtdump")
                            nc.vector.tensor_copy(out=wc_sb[:], in_=AD2[:])
                            nc.gpsimd.dma_start(out=dbg_wc[:, :], in_=wc_sb[:])
                        B2 = pp.tile([128, 384], F32, tag="pB", space="PSUM")
                        nc.tensor.matmul(B2[:], lhsT=W_sb[:, W2s(blk)], rhs=h1[:],
                                         start=True, stop=True)
                        h2 = sp.tile([128, 384], BF, tag="H2")
                        nc.scalar.activation(h2[:], B2[:],
                                             mybir.ActivationFunctionType.Relu,
                                             bias=B_sb[:, EB2(blk)])
                        C3 = pp.tile([128, 384], F32, tag="pC", space="PSUM")
                        nc.tensor.matmul(C3[:], lhsT=W_sb[:, W3p(blk)], rhs=h2[:],
                                         start=True, stop=True)
                        scl = sp.tile([128, 384], F32, tag="scl")
                        ln_tail(C3, EC3(blk), scl, 384)
                        newef = sp.tile([128, 384], F32, tag="newef")
                        nc.vector.tensor_tensor(out=newef[:], in0=scl[:],
                                                in1=ef3[:], op=mybir.AluOpType.add)
                        nc.sync.dma_start(
                            out=ef_dram.ap().rearrange("(e p) f -> p e f", p=128)[
                                :, bass.ds(b * BT + g * 3, 3), :],
                            in_=newef[:].rearrange("p (e f) -> p e f", f=128))
                        neb = sp.tile([128, 384], BF, tag="neb")
                        nc.vector.tensor_copy(out=neb[:], in_=newef[:])
                        M3 = sp.tile([128, 384], BF, tag="M3")
                        for t in range(3):
                            tt = g * 3 + t
                            ts = slice(t * 128, (t + 1) * 128)
                            nc.vector.tensor_tensor(
                                out=M3[:, ts],
                                in0=drel[:, tt:tt + 1].to_broadcast([128, 128]),
                                in1=io_sb[:, :],
                                op=mybir.AluOpType.is_equal)
                        for t in range(3):
                            ts = slice(t * 128, (t + 1) * 128)
                            last = (g == BT // 3 - 1) and (t == 2)
                            nc.tensor.matmul(aggE[:], lhsT=neb[:, ts],
                                             rhs=M3[:, ts],
                                             start=(g == 0 and t == 0), stop=last)
                    if dbg and blk == 0 and ib == 0:
                        agg_sb = sp.tile([128, 128], F32, tag="aggdump")
                        nc.vector.tensor_copy(out=agg_sb[:], in_=aggE[:])
                        nc.sync.dma_start(out=dbg_agg[:, :], in_=agg_sb[:])
                    nn = node_mlp(bass.ds(b * 128, 128), blk, aggE)
                    if blk < P_BLOCKS - 1:
                        make_tables(nn, b * 128, blk + 1)
                    else:
                        decoder(nn, bass.ds(b * 128, 128))

        if dbg:
            nc.sync.dma_start(out=dbg_nf1[:, :], in_=nf_sb[:])
            ee2_ = sp.tile([128, 9 * 128], F32, tag="edump")
            nc.sync.dma_start(out=ee2_[:], in_=ef_dram.ap().rearrange(
                "(e p) f -> p e f", p=128)[:, 91 * BT:91 * BT + 9, :])
            nc.sync.dma_start(out=dbg_ef1[:, :], in_=ee2_[:])
        nc.sync.dma_start(out=out_col[:, :], in_=oc_sb[:])

    nc.compile()
    return nc


# --------------------------------------------------------------------------
# host side
# --------------------------------------------------------------------------

def _np(x):
    return np.asarray(x, dtype=np.float32)


def preprocess(node_features, edge_features, src, dst):
    src = np.asarray(src).astype(np.int64)
    dst = np.asarray(dst).astype(np.int64)
    nf = _np(node_features)
    efe = _np(edge_features)

    core = dst // NPC_RAW
    dloc = dst - core * NPC_RAW

    order = np.lexsort((dloc, core))
    core_s, dloc_s, src_s = core[order], dloc[order], src[order]
    counts = np.zeros((NCORES, NB), dtype=np.int64)
    np.add.at(counts, (core_s, dloc_s // 128), 1)
    BT = int(np.ceil(counts.max() / 128))
    BT = max(3, ((BT + 2) // 3) * 3)
    ECP = NB * BT * 128

    srcmap_g = ((src_s // NPC_RAW) * NPC + (src_s % NPC_RAW)).astype(np.int32)

    per_core = []
    for c in range(NCORES):
        sel = core_s == c
        dl = dloc_s[sel].astype(np.int32)
        sm = srcmap_g[sel]
        od = order[sel]
        blk = dl // 128
        srcmap_c = np.zeros(ECP, dtype=np.int32)
        dstloc_c = np.zeros(ECP, dtype=np.int32)
        dstrel_c = np.full(ECP, -1.0, dtype=np.float32)
        efeat_c = np.zeros((ECP, IN_E), dtype=np.float32)
        ptr = 0
        for b in range(NB):
            nb_cnt = int(np.searchsorted(blk, b + 1) - np.searchsorted(blk, b))
            s = slice(ptr, ptr + nb_cnt)
            bs = b * BT * 128
            srcmap_c[bs:bs + nb_cnt] = sm[s]
            dstloc_c[bs:bs + nb_cnt] = dl[s]
            dstrel_c[bs:bs + nb_cnt] = (dl[s] - b * 128).astype(np.float32)
            efeat_c[bs:bs + nb_cnt] = efe[od[s]]
            ptr += nb_cnt
        nfeat_c = np.zeros((NPC, IN_N), dtype=np.float32)
        nfeat_c[:NPC_RAW] = nf[c * NPC_RAW:(c + 1) * NPC_RAW]
        per_core.append(dict(nfeat=nfeat_c, efeat=efeat_c, srcmap=srcmap_c,
                             dstloc=dstloc_c, dstrel=dstrel_c))
    return per_core, BT


def pack_weights(params):
    NSLOT = P_BLOCKS * 9 + 4 + 3
    Wmm = np.zeros((128, NSLOT * 128), dtype=np.float32)
    Bias = np.zeros((128, 100), dtype=np.float32)
    J = np.eye(H, dtype=np.float64) - 1.0 / H

    def fold(W3, b3):
        W3 = np.asarray(W3, np.float64)
        b3 = np.asarray(b3, np.float64)
        return (W3 @ J).astype(np.float32), (b3 - b3.mean()).astype(np.float32)

    def check_ln(ln):
        g, b = ln
        assert np.allclose(_np(g), 1.0, atol=1e-6) and \
            np.allclose(_np(b), 0.0, atol=1e-6), "nontrivial LN unsupported"

    def put(i, W):
        W = _np(W)
        Wmm[:W.shape[0], i * 128:i * 128 + W.shape[1]] = W

    for k in range(P_BLOCKS):
        eb = params["edge_blocks"][k]
        nbk = params["node_blocks"][k]
        check_ln(eb["ln"]); check_ln(nbk["ln"])
        (W1, b1), (W2, b2), (W3, b3) = eb["layers"]
        W1 = _np(W1)
        W3p, c3 = fold(W3, b3)
        put(k * 9 + 0, W1[0:128])
        put(k * 9 + 1, W1[128:256])
        put(k * 9 + 2, W1[256:384])
        put(k * 9 + 3, W2)
        put(k * 9 + 4, W3p)
        Bias[:, k * 6 + 0] = _np(b1)
        Bias[:, k * 6 + 1] = _np(b2)
        Bias[:, k * 6 + 2] = c3
        (V1, a1), (V2, a2), (V3, a3) = nbk["layers"]
        V1 = _np(V1)
        V3p, cn3 = fold(V3, a3)
        put(k * 9 + 5, V1[0:128])
        put(k * 9 + 6, V1[128:256])
        put(k * 9 + 7, V2)
        put(k * 9 + 8, V3p)
        Bias[:, k * 6 + 3] = _np(a1)
        Bias[:, k * 6 + 4] = _np(a2)
        Bias[:, k * 6 + 5] = cn3
    base = P_BLOCKS * 9
    bb = P_BLOCKS * 6
    ee = params["edge_enc"]; ne = params["node_enc"]
    check_ln(ee["ln"]); check_ln(ne["ln"])
    (F1, g1), (F2_, g2), (F3_, g3) = ee["layers"]
    F3p, ce3 = fold(F3_, g3)
    put(base + 0, F2_)
    put(base + 1, F3p)
    (N1, h1), (N2_, h2), (N3_, h3) = ne["layers"]
    N3p, cn3e = fold(N3_, h3)
    put(base + 2, N2_)
    put(base + 3, N3p)
    (D1, e1), (D2_, e2), (D3_, e3) = params["decoder"]["layers"]
    put(base + 4, D1)
    put(base + 5, D2_)
    put(base + 6, D3_)
    Bias[:, bb + 0] = _np(g1); Bias[:, bb + 1] = _np(g2); Bias[:, bb + 2] = ce3
    Bias[:, bb + 3] = _np(h1); Bias[:, bb + 4] = _np(h2); Bias[:, bb + 5] = cn3e
    Bias[:, bb + 6] = _np(e1); Bias[:, bb + 7] = _np(e2)
    Bias[:OUT, bb + 8] = _np(e3)
    Bias[:, 99] = EPS

    return dict(
        Wmm=Wmm.astype(BF16),
        We1=_np(F1).astype(BF16),
        Wn1=_np(N1).astype(BF16),
        Bias=Bias,
        ident=np.eye(128, dtype=np.float32).astype(BF16),
        iota=np.ascontiguousarray(
            np.arange(128, dtype=np.float32)[None, :].repeat(128, 0)),
    )


def get_runner(BT, nblocks=P_BLOCKS, dbg=False):
    key = ("nc", BT, nblocks, dbg)
    if key not in _CACHE:
        nc = build_nc(BT, nblocks, dbg)
        from runner_embedded import make_runner
        _CACHE[key] = make_runner(nc, NCORES)
    return _CACHE[key]


def kernel(node_features, edge_features, params, src, dst):
    per_core, BT = preprocess(node_features, edge_features, src, dst)
    wts = pack_weights(params)
    r = get_runner(BT)
    in_maps = []
    for c in range(NCORES):
        m = dict(per_core[c])
        m.update(wts)
        in_maps.append(m)
    res = r.run_np(in_maps)
    out = np.empty((N, OUT), dtype=np.float32)
    for c in range(NCORES):
        oc = res[c]["out_col"]
        out[c * NPC_RAW:(c + 1) * NPC_RAW] = oc[:, :NPC_RAW].T
    return out


# --------------------------------------------------------------------------
# embedded runner (kernel.py must be self-contained)
# --------------------------------------------------------------------------
import sys as _sys
import types as _types

_runner_src = '''
import time
import numpy as np
import jax
from jax.sharding import Mesh, PartitionSpec, NamedSharding
from jax.experimental.shard_map import shard_map
import concourse.mybir as mybir
from concourse.bass2jax import _bass_exec_p, partition_id_tensor, install_neuronx_cc_hook


def make_runner(nc, n_cores=8):
    install_neuronx_cc_hook()
    if not nc.is_finalized():
        nc.finalize()
    partition_name = nc.partition_id_tensor.name if nc.partition_id_tensor else None
    in_names, out_names, out_avals, zero_outs = [], [], [], []
    for alloc in nc.m.functions[0].allocations:
        if not isinstance(alloc, mybir.MemoryLocationSet):
            continue
        name = alloc.memorylocations[0].name
        if alloc.kind == "ExternalInput":
            if name != partition_name:
                in_names.append(name)
        elif alloc.kind == "ExternalOutput":
            out_names.append(name)
            shape = tuple(alloc.tensor_shape)
            dtype = mybir.dt.np(alloc.dtype)
            out_avals.append(jax.core.ShapedArray(shape, dtype))
            zero_outs.append(np.zeros(shape, dtype))
    n_params = len(in_names)
    n_outs = len(out_avals)
    all_in_names = list(in_names) + list(out_names)
    if partition_name is not None:
        all_in_names.append(partition_name)
    donate = tuple(range(n_params, n_params + n_outs))

    def _body(*args):
        operands = list(args)
        if partition_name is not None:
            operands.append(partition_id_tensor())
        outs = _bass_exec_p.bind(
            *operands, out_avals=tuple(out_avals), in_names=tuple(all_in_names),
            out_names=tuple(out_names), lowering_input_output_aliases=(),
            sim_require_finite=True, sim_require_nnan=True, nc=nc)
        return tuple(outs)

    devices = jax.devices()[:n_cores]
    mesh = Mesh(np.asarray(devices), ("core",))
    in_specs = (PartitionSpec("core"),) * (n_params + n_outs)
    out_specs = (PartitionSpec("core"),) * n_outs
    sharded = jax.jit(
        shard_map(_body, mesh=mesh, in_specs=in_specs, out_specs=out_specs,
                  check_rep=False),
        donate_argnums=donate, keep_unused=True)
    sharding = NamedSharding(mesh, PartitionSpec("core"))

    class Runner:
        def put_inputs(self, in_maps):
            return [jax.device_put(
                np.concatenate([np.asarray(m[n]) for m in in_maps], axis=0),
                sharding) for n in in_names]

        def zeros(self):
            return [jax.device_put(
                np.zeros((n_cores * z.shape[0], *z.shape[1:]), z.dtype), sharding)
                for z in zero_outs]

        def run(self, in_arrs):
            outs = sharded(*in_arrs, *self.zeros())
            jax.block_until_ready(outs)
            return outs

        def run_np(self, in_maps):
            outs = self.run(self.put_inputs(in_maps))
            return [{n: np.asarray(outs[i]).reshape(n_cores, *out_avals[i].shape)[c]
                     for i, n in enumerate(out_names)} for c in range(n_cores)]

        def time_runs(self, in_arrs, iters=5):
            ts = []
            for _ in range(iters):
                t0 = time.perf_counter()
                outs = sharded(*in_arrs, *self.zeros())
                jax.block_until_ready(outs)
                ts.append(time.perf_counter() - t0)
            return ts

    return Runner()
'''

_mod = _types.ModuleType("runner_embedded")
exec(_runner_src, _mod.__dict__)
_sys.modules["runner_embedded"] = _mod
